# revision 41
# baseline (speedup 1.0000x reference)
"""Trainium2 Bass kernel for nn_CapsuleLayer (capsule dynamic routing).

Math (reference):
    u_hat[b,c,u,s] = sum_i W[c,u,s,i] * x[b,i,c]          (never materialized)
    3 routing iterations:
        c_ij = softmax_u(b_ij)                            [C, U]
        s_j[b,u,s]  = sum_c c_ij[c,u] * u_hat[b,c,u,s]
        v_j = squash(s_j)   (norm over the U axis!)
        u_vj1[c,u] = sum_{b,s} u_hat[b,c,u,s] v_j[b,u,s] / B
        b_ij += u_vj1
    output = v_j  (B, U, S, 1)

Sharding: channels C=1152 split 8 ways (CL=144/core); x and W shards live in
SBUF, u_hat recomputed on the fly as matrix products.  Cross-core combine of
the s_j partial sums: f16 AllReduce per iteration (measured ~20us faster than
AllGather + on-chip reduction), ReduceScatter for the final iteration (each
core only needs its 1/8 of batch rows for the output).

Engineering notes:
  - column order is (u,s): col = u*32 + s, so the routing-update reduce over
    s is contiguous; squash's reduce over u is the strided one (it is 9x
    smaller).
  - a dummy Exp after each squash pulls the exp-table LoadActFuncSet into an
    Act-idle window instead of stalling the first softmax mid-routing.
  - softmax skips the max-subtraction (logit magnitudes are bounded ~10).
  - PE instruction stream is interleaved (mm2 groups, em, mm1 groups) and a
    chain of tiny warm-up matmuls fires when the AllReduce result lands, so
    the tensor engine keeps its p-state through the squash window.

Per-core layouts (host-prepared):
    XT [128, T*B ] f16 : XT[p, t*256+b]      = x[b, i, c],  ci = 128t+p
    XF [128, 2*KCI] f16: XF[p, bc*2304+ci]   = x[b, i, c],  b  = 128bc+p
    WM [128, T*US] f16 : WM[p, t*320+u*32+s] = W[c, u, s, i], ci = 128t+p
    EM [128, 128] f32  : block-diag selector, EM[p,m] = (p//16==m//16)/256
    ID [128, 128] f16  : identity (PE-side shard accumulation)
"""

import numpy as np

B, IN_U, C, NUM_U, S = 256, 16, 1152, 10, 32
NCORES = 8
CL = C // NCORES          # 144 channels per core
KCI = CL * IN_U           # 2304 contraction size
T = KCI // 128            # 18 partition chunks
US = NUM_U * S            # 320
NITER = 3
G = 3                     # routing groups
CPG = T // G              # 6 chunks per group
LOADH = 6                 # input load chunks
TPH = T // LOADH
PR = 128 // NCORES        # 16 partition rows per core after ReduceScatter

_CACHE = {}


def _build_program(bypass_cc=False, reps=1, cc_mode="ag"):
    import concourse.bacc as bacc
    import concourse.tile as tile
    from concourse import mybir
    from contextlib import ExitStack

    f32 = mybir.dt.float32
    f16 = mybir.dt.float16
    AX = mybir.AxisListType
    ALU = mybir.AluOpType
    AF = mybir.ActivationFunctionType

    nc = bacc.Bacc(None, num_devices=NCORES)
    xt_d = nc.declare_dram_parameter("xt", [128, T * B], f16, isOutput=False)
    xf_d = nc.declare_dram_parameter("xf", [128, 2 * KCI], f16, isOutput=False)
    wm_d = nc.declare_dram_parameter("wm", [128, T * US], f16, isOutput=False)
    em_d = nc.declare_dram_parameter("em", [128, 128], f32, isOutput=False)
    id_d = nc.declare_dram_parameter("idm", [128, 128], f16, isOutput=False)
    out_d = nc.declare_dram_parameter("out", [PR, 2 * US], f32, isOutput=True)

    with tile.TileContext(nc) as tc, ExitStack() as octx:
        for _ in range(reps):
            with ExitStack() as ctx:
                _emit_body(
                    nc, tc, ctx, mybir, xt_d, xf_d, wm_d, em_d, id_d, out_d,
                    bypass_cc, f32, f16, AX, ALU, AF, cc_mode,
                )
    return nc


def _emit_body(nc, tc, ctx, mybir, xt_d, xf_d, wm_d, em_d, id_d, out_d,
               bypass_cc, f32, f16, AX, ALU, AF, cc_mode="ag"):
    singles = ctx.enter_context(tc.tile_pool(name="singles", bufs=1))
    big = ctx.enter_context(tc.tile_pool(name="big", bufs=2))
    work = ctx.enter_context(tc.tile_pool(name="work", bufs=2))
    shards = ctx.enter_context(tc.tile_pool(name="shards", bufs=2))
    psum_s = ctx.enter_context(tc.tile_pool(name="psum_s", bufs=1, space="PSUM"))
    psum_m = ctx.enter_context(tc.tile_pool(name="psum_m", bufs=3, space="PSUM"))
    psum_u = ctx.enter_context(tc.tile_pool(name="psum_u", bufs=1, space="PSUM"))
    psum_g = ctx.enter_context(tc.tile_pool(name="psum_g", bufs=1, space="PSUM"))
    dram = ctx.enter_context(tc.tile_pool(name="dram", bufs=2, space="DRAM"))

    # ---- input loads (paired xt/wm chunks so mm1 can start early; first
    # chunks are single-t so the very first matmuls fire ~1us in) ----
    xt_sb = singles.tile([128, T * B], f16, name="xt_sb")
    wm_sb = singles.tile([128, T * US], f16, name="wm_sb")
    bounds = [0, 1, 2, 3, 6, 9, 12, 15, 18]
    for h in range(len(bounds) - 1):
        lo, hi = bounds[h], bounds[h + 1]
        nc.sync.dma_start(
            out=xt_sb[:, lo * B : hi * B], in_=xt_d[:, lo * B : hi * B]
        )
        nc.sync.dma_start(
            out=wm_sb[:, lo * US : hi * US], in_=wm_d[:, lo * US : hi * US]
        )
    em_sb = singles.tile([128, 128], f32, name="em_sb")
    nc.sync.dma_start(out=em_sb, in_=em_d[:])
    id_sb = singles.tile([128, 128], f16, name="id_sb")
    nc.sync.dma_start(out=id_sb, in_=id_d[:])
    xf_sb = singles.tile([128, 2 * KCI], f16, name="xf_sb")
    for bc in range(2):
        nc.sync.dma_start(
            out=xf_sb[:, bc * KCI : (bc + 1) * KCI],
            in_=xf_d[:, bc * KCI : (bc + 1) * KCI],
        )

    bij_sb = singles.tile([128, T * NUM_U], f32, name="bij_sb")
    wm_g = [wm_sb[:, g * CPG * US : (g + 1) * CPG * US] for g in range(G)]

    def mm1_matmuls(rhs_groups, ps, ts):
        """Accumulate s_partial for chunk list ts into ps[bc] (PSUM)."""
        for t in ts:
            rhs = rhs_groups[t // CPG]
            tl = t % CPG
            for bc in range(2):
                nc.tensor.matmul(
                    ps[bc],
                    lhsT=xt_sb[:, t * B + bc * 128 : t * B + bc * 128 + 128],
                    rhs=rhs[:, tl * US : (tl + 1) * US],
                    start=(t == 0),
                    stop=(t == T - 1),
                )

    def mm1_copy(ps, scale):
        cc_sb = work.tile([128, 2 * US], f16, name="cc_sb")
        for bc in range(2):
            nc.scalar.activation(
                out=cc_sb[:, bc * US : (bc + 1) * US],
                in_=ps[bc],
                func=AF.Copy,
                scale=float(scale),
            )
        return cc_sb

    def allgather(cc_sb):
        """AllGather the partial sums; accumulate the 8 shards on the PE
        into PSUM (f32).  Returns (s_ps0, s_ps1) [128, US] each."""
        cc_in = dram.tile([128, 2 * US], f16, name="cc_in")
        nc.sync.dma_start(out=cc_in, in_=cc_sb)
        ag_out = dram.tile(
            [128 * NCORES, 2 * US], f16, name="ag_out", addr_space="Shared"
        )
        if bypass_cc:
            nc.gpsimd.dma_start(
                out=ag_out.rearrange("(k p) c -> k p c", k=NCORES),
                in_=cc_in.unsqueeze(0).broadcast_to([NCORES, 128, 2 * US]),
            )
        else:
            nc.gpsimd.collective_compute(
                "AllGather",
                ALU.bypass,
                replica_groups=[list(range(NCORES))],
                ins=[cc_in.opt()],
                outs=[ag_out.opt()],
            )
        ps = [
            psum_g.tile([128, US], f32, name="ag_ps0"),
            psum_g.tile([128, US], f32, name="ag_ps1"),
        ]
        pks = []
        for j in range(NCORES // 2):  # paired shard loads: 4 DMAs of [128,1280]
            p_j = shards.tile([128, 4 * US], f16, name=f"p_{j}")
            nc.sync.dma_start(
                out=p_j.rearrange("p (k c) -> p k c", k=2),
                in_=ag_out[2 * j * 128 : (2 * j + 2) * 128, :].rearrange(
                    "(k p) c -> p k c", k=2
                ),
            )
            pks.append(p_j)
        # half-major: finish s_ps0 first so squash/v16 of half 0 can start
        # while half 1's accumulation still runs
        for half in range(2):
            for k in range(NCORES):
                nc.tensor.matmul(
                    ps[half],
                    lhsT=id_sb,
                    rhs=pks[k // 2][
                        :, (k % 2) * 2 * US + half * US : (k % 2) * 2 * US
                        + (half + 1) * US
                    ],
                    start=(k == 0),
                    stop=(k == NCORES - 1),
                )
        return ps

    def squash_factor(sq, rows):
        """fct[p,(bc,s)] = mag/(1+mag^2) from sq = s^2; sqrt via exp/ln."""
        magsq = work.tile([rows, 2 * S], f32, name="magsq")
        nc.vector.reduce_sum(
            out=magsq,
            in_=sq.rearrange("p (bc u s) -> p bc u s", bc=2, u=NUM_U).transpose(
                [0, 1, 3, 2]
            ),
            axis=AX.X,
        )
        mag = work.tile([rows, 2 * S], f32, name="mag")
        nc.scalar.sqrt(out=mag, in_=magsq)
        den = work.tile([rows, 2 * S], f32, name="den")
        nc.scalar.add(out=den, in_=magsq, add=1.0)
        rden = work.tile([rows, 2 * S], f32, name="rden")
        nc.vector.reciprocal(out=rden, in_=den)
        fct = work.tile([rows, 2 * S], f32, name="fct")
        nc.vector.tensor_mul(out=fct, in0=mag, in1=rden)
        return fct

    def squash_psum(ps):
        """squash from the PE-accumulated PSUM halves -> v16 [128, 640] f16."""
        sq = work.tile([128, 2 * US], f32, name="sq")
        for half in range(2):
            nc.scalar.square(
                out=sq[:, half * US : (half + 1) * US], in_=ps[half]
            )
        fct = squash_factor(sq, 128)
        v16 = work.tile([128, 2 * US], f16, name="v16")
        for half in range(2):
            nc.vector.tensor_mul(
                out=v16[:, half * US : (half + 1) * US].rearrange(
                    "p (u s) -> p u s", u=NUM_U
                ),
                in0=ps[half].rearrange("p (u s) -> p u s", u=NUM_U),
                in1=fct[:, half * S : (half + 1) * S]
                .unsqueeze(1)
                .broadcast_to([128, NUM_U, S]),
            )
        return v16

    def allreduce(cc_sb):
        cc_in = dram.tile([128, 2 * US], f16, name="ar_in")
        cc_out = dram.tile([128, 2 * US], f16, name="ar_out")
        nc.sync.dma_start(out=cc_in, in_=cc_sb)
        if bypass_cc:
            nc.gpsimd.dma_start(out=cc_out, in_=cc_in)
        else:
            nc.gpsimd.collective_compute(
                "AllReduce",
                ALU.add,
                replica_groups=[list(range(NCORES))],
                ins=[cc_in.opt()],
                outs=[cc_out.opt()],
            )
        s_sb = work.tile([128, 2 * US], f16, name="s_sb")
        nc.sync.dma_start(out=s_sb, in_=cc_out)
        # PE warm-up: a chain of tiny matmuls fires the moment the AR result
        # lands, bridging the squash window so mm2 starts at a hot p-state
        # (PE is otherwise idle ~12us during the collective and downclocks).
        warm = psum_g.tile([128, 16], f32, name="warm")
        for _ in range(12):
            nc.tensor.matmul(
                warm, lhsT=id_sb, rhs=s_sb[:, 0:16], start=True, stop=True
            )
        return s_sb

    def squash_sbuf(s_sb):
        """squash from an SBUF f16 s tile -> v16 [128, 640] f16."""
        sq = work.tile([128, 2 * US], f32, name="sq")
        nc.scalar.square(out=sq, in_=s_sb)
        fct = squash_factor(sq, 128)
        # dummy Exp: forces the exp-table LoadActFuncSet to execute HERE,
        # in the Act-idle window alongside the DVE factor chain, instead of
        # stalling the first real softmax Exp mid-routing.
        dummy = work.tile([1, 2], f32, name="dummy_exp")
        nc.scalar.activation(out=dummy, in_=fct[0:1, 0:2], func=AF.Exp)
        v16 = work.tile([128, 2 * US], f16, name="v16")
        # per-half writes: mm2's first (bc0) matmul unblocks after the first
        for bc in range(2):
            nc.vector.tensor_mul(
                out=v16[:, bc * US : (bc + 1) * US].rearrange(
                    "p (u s) -> p u s", u=NUM_U
                ),
                in0=s_sb[:, bc * US : (bc + 1) * US].rearrange(
                    "p (u s) -> p u s", u=NUM_U
                ),
                in1=fct[:, bc * S : (bc + 1) * S]
                .unsqueeze(1)
                .broadcast_to([128, NUM_U, S]),
            )
        return v16

    def reduce_scatter(cc_sb):
        cc_in = dram.tile([128, 2 * US], f16, name="rs_in")
        cc_out = dram.tile([PR, 2 * US], f16, name="rs_out")
        nc.sync.dma_start(out=cc_in, in_=cc_sb)
        if bypass_cc:
            nc.gpsimd.dma_start(out=cc_out, in_=cc_in[0:PR, :])
        else:
            nc.gpsimd.collective_compute(
                "ReduceScatter",
                ALU.add,
                replica_groups=[list(range(NCORES))],
                ins=[cc_in.opt()],
                outs=[cc_out.opt()],
            )
        s16 = work.tile([PR, 2 * US], f16, name="s16")
        nc.sync.dma_start(out=s16, in_=cc_out)
        return s16

    def squash_final(s16):
        sq = work.tile([PR, 2 * US], f32, name="fsq")
        nc.scalar.square(out=sq, in_=s16)
        fct = squash_factor(sq, PR)
        v32 = work.tile([PR, 2 * US], f32, name="v32")
        nc.vector.tensor_mul(
            out=v32.rearrange("p (bc u s) -> p bc u s", bc=2, u=NUM_U),
            in0=s16.rearrange("p (bc u s) -> p bc u s", bc=2, u=NUM_U),
            in1=fct.rearrange("p (bc s) -> p bc s", bc=2)
            .unsqueeze(2)
            .broadcast_to([PR, 2, NUM_U, S]),
        )
        return v32

    def mm2_mm(g, v16):
        """M chunk matmuls for group g (PE + Act PSUM->SBUF copies)."""
        m_g = big.tile([128, CPG * US], f16, name=f"m_g{g}")
        for tl in range(CPG):
            t = g * CPG + tl
            ps = psum_m.tile([128, US], f32, name="m_ps")
            for bc in range(2):
                nc.tensor.matmul(
                    ps,
                    lhsT=xf_sb[:, bc * KCI + t * 128 : bc * KCI + (t + 1) * 128],
                    rhs=v16[:, bc * US : (bc + 1) * US],
                    start=(bc == 0),
                    stop=(bc == 1),
                )
            nc.scalar.copy(out=m_g[:, tl * US : (tl + 1) * US], in_=ps)
        return m_g

    def mm2_qr(g, m_g):
        """q = W*M (split Pool/DVE) and r = sum_s q for group g."""
        q_g = big.tile([128, CPG * US], f16, name=f"q_g{g}")
        # DVE takes the FIRST 4 chunks (their PSUM->SBUF copies land first,
        # so its multiply starts ~0.9us earlier); Pool (~3.9x slower per
        # element) takes the last 2.  The s-reduce is split the same way so
        # each part fires as soon as its half of q exists instead of waiting
        # for the whole tile.
        dc = 4 * US
        nc.vector.tensor_mul(
            out=q_g[:, :dc], in0=wm_g[g][:, :dc], in1=m_g[:, :dc]
        )
        nc.gpsimd.tensor_mul(
            out=q_g[:, dc:], in0=wm_g[g][:, dc:], in1=m_g[:, dc:]
        )
        r_g = work.tile([128, CPG * NUM_U], f32, name=f"r_g{g}")
        nc.vector.reduce_sum(
            out=r_g[:, : 4 * NUM_U],
            in_=q_g[:, :dc].rearrange("p (t u s) -> p t u s", t=4, u=NUM_U),
            axis=AX.X,
        )
        nc.vector.reduce_sum(
            out=r_g[:, 4 * NUM_U :],
            in_=q_g[:, dc:].rearrange("p (t u s) -> p t u s", t=2, u=NUM_U),
            axis=AX.X,
        )
        return r_g

    def em_group(g, r_g, u_ps, first):
        ups = u_ps[:, g * CPG * NUM_U : (g + 1) * CPG * NUM_U]
        nc.tensor.matmul(ups, lhsT=em_sb, rhs=r_g, start=True, stop=True)
        return ups

    def softmax_bm_group(g, ups, first):
        gc = slice(g * CPG * NUM_U, (g + 1) * CPG * NUM_U)
        if first:
            nc.vector.tensor_copy(out=bij_sb[:, gc], in_=ups)
        else:
            nc.vector.tensor_add(out=bij_sb[:, gc], in0=bij_sb[:, gc], in1=ups)
        ex = work.tile([128, CPG * NUM_U], f32, name=f"ex{g}")
        nc.scalar.activation(out=ex, in_=bij_sb[:, gc], func=AF.Exp)
        sm = work.tile([128, CPG], f32, name=f"sm{g}")
        nc.vector.reduce_sum(
            out=sm, in_=ex.rearrange("p (t u) -> p t u", t=CPG), axis=AX.X
        )
        rsm = work.tile([128, CPG], f32, name=f"rsm{g}")
        nc.vector.reciprocal(out=rsm, in_=sm)
        cij = work.tile([128, CPG * NUM_U], f16, name=f"cij{g}")
        nc.vector.tensor_mul(
            out=cij.rearrange("p (t u) -> p t u", t=CPG),
            in0=ex.rearrange("p (t u) -> p t u", t=CPG),
            in1=rsm[:].unsqueeze(2).broadcast_to([128, CPG, NUM_U]),
        )
        bm = big.tile([128, CPG * US], f16, name=f"bm{g}")
        # split the wm*c broadcast multiply across DVE/Pool: DVE (fast) takes
        # the FIRST chunks since mm1 consumes bm in t order — its first
        # matmuls of the group unblock ~1us earlier than with Pool leading
        hc = CPG // 2
        nc.vector.tensor_mul(
            out=bm[:, : hc * US].rearrange("p (t u s) -> p t u s", t=hc, u=NUM_U),
            in0=wm_g[g][:, : hc * US].rearrange(
                "p (t u s) -> p t u s", t=hc, u=NUM_U
            ),
            in1=cij[:, : hc * NUM_U]
            .rearrange("p (t u) -> p t u", t=hc)
            .unsqueeze(3)
            .broadcast_to([128, hc, NUM_U, S]),
        )
        nc.gpsimd.tensor_mul(
            out=bm[:, hc * US :].rearrange(
                "p (t u s) -> p t u s", t=CPG - hc, u=NUM_U
            ),
            in0=wm_g[g][:, hc * US :].rearrange(
                "p (t u s) -> p t u s", t=CPG - hc, u=NUM_U
            ),
            in1=cij[:, hc * NUM_U :]
            .rearrange("p (t u) -> p t u", t=CPG - hc)
            .unsqueeze(3)
            .broadcast_to([128, CPG - hc, NUM_U, S]),
        )
        return bm

    def routing_and_mm1(v16, first, u_ps):
        """Full routing update fused with the next mm1.  Emission order keeps
        the PE stream hot and the DVE chain for group g's softmax/bm ahead of
        group g+1's elementwise work, so mm1(g0) can start while mm2(g2)'s
        elementwise tail is still running."""
        m0 = mm2_mm(0, v16)
        r0 = mm2_qr(0, m0)
        m1 = mm2_mm(1, v16)
        ups0 = em_group(0, r0, u_ps, first)
        r1 = mm2_qr(1, m1)
        bm0 = softmax_bm_group(0, ups0, first)
        m2 = mm2_mm(2, v16)
        ups1 = em_group(1, r1, u_ps, first)
        bm1 = softmax_bm_group(1, ups1, first)
        r2 = mm2_qr(2, m2)
        ps = [
            psum_s.tile([128, US], f32, name="s_ps0"),
            psum_s.tile([128, US], f32, name="s_ps1"),
        ]
        groups = [bm0, bm1, None]
        mm1_matmuls(groups, ps, range(0, 2 * CPG))
        ups2 = em_group(2, r2, u_ps, first)
        bm2 = softmax_bm_group(2, ups2, first)
        groups[2] = bm2
        mm1_matmuls(groups, ps, range(2 * CPG, T))
        return ps

    # ---------------- main schedule ----------------
    # iteration 0: c_ij is uniform = 1/NUM_U
    def combine(cc):
        if cc_mode == "ar":
            return squash_sbuf(allreduce(cc))
        return squash_psum(allgather(cc))

    u_ps = psum_u.tile([128, T * NUM_U], f32, name="u_ps")
    ps = [
        psum_s.tile([128, US], f32, name="s_ps0"),
        psum_s.tile([128, US], f32, name="s_ps1"),
    ]
    mm1_matmuls([wm_g[g] for g in range(G)], ps, range(T))
    cc = mm1_copy(ps, 1.0 / NUM_U)
    v16 = combine(cc)

    for it in range(1, NITER):
        ps = routing_and_mm1(v16, first=(it == 1), u_ps=u_ps)
        cc = mm1_copy(ps, 1.0)
        if it < NITER - 1:
            v16 = combine(cc)
        else:
            s16 = reduce_scatter(cc)
            v32 = squash_final(s16)
            nc.sync.dma_start(out=out_d[:], in_=v32)


def _prep_core_inputs(x, W, core, em, idm):
    sl = slice(core * CL, (core + 1) * CL)
    xs = np.ascontiguousarray(x[:, :, sl])  # (B, I, CL)
    ws = np.ascontiguousarray(W[0, sl])     # (CL, U, S, I)
    xt = xs.transpose(2, 1, 0).reshape(T, 128, B)
    xt = np.ascontiguousarray(xt.transpose(1, 0, 2)).reshape(128, T * B)
    xf = xs.transpose(0, 2, 1).reshape(2, 128, KCI)
    xf = np.ascontiguousarray(xf.transpose(1, 0, 2)).reshape(128, 2 * KCI)
    wm = ws.transpose(0, 3, 1, 2).reshape(T, 128, US)  # (c,i,u,s), col=u*32+s
    wm = np.ascontiguousarray(wm.transpose(1, 0, 2)).reshape(128, T * US)
    return {
        "xt": xt.astype(np.float16),
        "xf": xf.astype(np.float16),
        "wm": wm.astype(np.float16),
        "em": em,
        "idm": idm,
    }


def prep_in_maps(x, W):
    x = np.asarray(x, dtype=np.float32)
    W = np.asarray(W, dtype=np.float32)
    em = (np.kron(np.eye(8, dtype=np.float32), np.ones((16, 16), np.float32))
          / float(B))
    idm = np.eye(128, dtype=np.float16)
    return [_prep_core_inputs(x, W, core, em, idm) for core in range(NCORES)]


def postprocess(results):
    """Assemble per-core ReduceScatter shards (16 partition rows each) into
    the full [128, 640] (col = bc*320 + u*32 + s), then -> (B, U, S, 1)."""
    full = np.concatenate(
        [np.asarray(results[r]["out"], np.float32) for r in range(NCORES)],
        axis=0,
    )
    v = full.reshape(128, 2, NUM_U, S).transpose(1, 0, 2, 3)  # (bc,p,u,s)
    return np.ascontiguousarray(v.reshape(B, NUM_U, S)[..., None])


def get_program(reps=1):
    # AllReduce beats AllGather+on-chip reduce here by ~20us: the CCE does
    # the sum in the SDMA datapath, saving the 1.28MB gather drain + PE adds.
    key = f"nc{reps}"
    if key not in _CACHE:
        nc = _build_program(reps=reps, cc_mode="ar")
        nc.finalize()
        _CACHE[key] = nc
    return _CACHE[key]


def kernel(x, W):
    from concourse.bass_utils import run_bass_kernel_spmd

    nc = get_program()
    in_maps = prep_in_maps(x, W)
    res = run_bass_kernel_spmd(nc, in_maps, list(range(NCORES)))
    return postprocess(res.results)
